# revision 6
# baseline (speedup 1.0000x reference)
"""HQQLinearLoRA TRN2 kernel v4: out = x @ W + (x @ A) @ B + bias.

Data-parallel over tokens (16384) across 8 cores; per core
[2048, 4096] @ [4096, 4096] + rank-16 LoRA + bias.

v4 design (cost-model driven):
- Host passes x already transposed (xT [d, m_core]) and an identity
  matrix; no on-device DMA transposes at all.
- All loads are SWDGE cast-DMAs (f32 DRAM -> bf16 SBUF): the DMA cost
  model charges destination bytes, so bf16 destinations halve transfer
  time, and no DVE/Act conversion passes exist.
- x^T resident in SBUF as 9 chunk tiles [128, 32, mw] bf16 (128 KB/part
  total); W streamed once as quarter-n-tile tiles [128, 8, 512] bf16.
- x@A via ap_size=16 matmuls (output [128m, 16]): ~7ns each on PE vs
  53ns for the [16, m] orientation; one PE transpose (vs identity) per
  m-tile flips the result into p1t [17, m] for the fused K=17
  LoRA+bias matmul (row 16 of p1t memset to 1.0 once).
- ni=0 walks m-tiles one at a time riding x arrival, software
  pipelined: transpose(mt-1) sits between the k-walk halves of mt and
  B-mm(mt-1) after the walk, so the PE stream never waits on the DVE
  copies. ni>=1 walks groups of 4 m-tiles (4 psum banks + 4 draining),
  W(ni+1) quarters prefetched one per group. Final n-tile tapers
  4/4/4/2/1/1 so the drain pipeline empties under the last matmuls.
"""
import numpy as np
from contextlib import ExitStack

import concourse.bass as bass
import concourse.tile as tile
import concourse.mybir as mybir
from concourse import bacc
from concourse.bass_utils import run_bass_kernel_spmd

P = 128
NCORES = 8

B_DIM, S_DIM, D_DIM, R_DIM = 4, 4096, 4096, 16


def build_nc(m_core, d, r, n_tile=512):
    KT = d // P                 # 32 k-tiles
    MT = m_core // P            # 16 m-tiles
    NT = d // n_tile            # 8 n-tiles
    QK = 4                      # k-tiles per W piece
    NQ = KT // QK               # 8 pieces per W n-tile
    f32 = mybir.dt.float32
    bf16 = mybir.dt.bfloat16
    RB = r + 1                  # fused lora+bias contraction depth

    # x chunk widths (in m): fine first/last for fast lead-in
    widths = [P] + [2 * P] * 7 + [P]
    starts = np.cumsum([0] + widths).tolist()

    def chunk_of(mt):
        m0 = mt * P
        for c, (s, w) in enumerate(zip(starts, widths)):
            if s <= m0 < s + w:
                return c, m0 - s
        raise AssertionError

    nc = bacc.Bacc(target_bir_lowering=False)
    xT = nc.declare_dram_parameter("xT", [d, m_core], f32, isOutput=False)
    W = nc.declare_dram_parameter("W", [d, d], f32, isOutput=False)
    bias = nc.declare_dram_parameter("bias", [d], f32, isOutput=False)
    lora_A = nc.declare_dram_parameter("lora_A", [d, r], f32, isOutput=False)
    lora_B = nc.declare_dram_parameter("lora_B", [r, d], f32, isOutput=False)
    ident = nc.declare_dram_parameter("ident", [P, P], f32, isOutput=False)
    out = nc.declare_dram_parameter("out", [m_core, d], f32, isOutput=True)

    xTr = xT.rearrange("(kt p) m -> p kt m", p=P)
    Wr = W.rearrange("(kt p) n -> p kt n", p=P)

    with tile.TileContext(nc) as tc, ExitStack() as ctx:
        const = ctx.enter_context(tc.tile_pool(name="const", bufs=1))
        xtp = ctx.enter_context(tc.tile_pool(name="xtp", bufs=1))
        wrp = ctx.enter_context(tc.tile_pool(name="wrp", bufs=12))
        xasb = ctx.enter_context(tc.tile_pool(name="xasb", bufs=2))
        outstage = ctx.enter_context(tc.tile_pool(name="outstage", bufs=3))
        psum = ctx.enter_context(tc.tile_pool(name="psum", bufs=6, space="PSUM"))
        psxa = ctx.enter_context(tc.tile_pool(name="psxa", bufs=1, space="PSUM"))
        psxat = ctx.enter_context(tc.tile_pool(name="psxat", bufs=1, space="PSUM"))

        # ---- resident x^T chunk tiles (cast-DMA'd on demand)
        xt_c = [xtp.tile([P, KT, w], bf16, name=f"xt{c}")
                for c, w in enumerate(widths)]

        def load_x(c):
            s, w = starts[c], widths[c]
            nc.gpsimd.dma_start(xt_c[c][:], xTr[:, :, s:s + w])

        def xslice(mt, ki):
            c, off = chunk_of(mt)
            return xt_c[c][:, ki, off:off + P]

        # ---- W quarter tiles, cast-DMA'd; wq[ni % 2][q] holds current set
        wq = {}

        def load_wq(ni, q):
            t = wrp.tile([P, QK, n_tile], bf16, name="wq")
            nsl = slice(ni * n_tile, (ni + 1) * n_tile)
            nc.gpsimd.dma_start(t[:], Wr[:, q * QK:(q + 1) * QK, nsl])
            wq[(ni, q)] = t

        def wslice(ni, ki):
            return wq[(ni, ki // QK)][:, ki % QK, :]

        # ---- preamble: x chunk 0 first, then W0, then lora consts
        load_x(0)
        for q in range(NQ):
            load_wq(0, q)
        a_bf = const.tile([P, KT, r], bf16, name="a_bf")
        nc.gpsimd.dma_start(a_bf[:], lora_A.rearrange("(kt p) r -> p kt r", p=P))
        identb = const.tile([P, P], bf16, name="identb")
        nc.gpsimd.dma_start(identb[:], ident[:, :])
        load_x(1)
        b17 = const.tile([RB, d], bf16, name="b17")
        nc.gpsimd.dma_start(b17[0:r, :], lora_B[:, :])
        nc.gpsimd.dma_start(b17[r:r + 1, :], bias[:].unsqueeze(0))
        load_x(2)

        # ones row for the fused bias term: rows 0:16 are overwritten by
        # the per-m-tile copy2 writes before any B-matmul reads them.
        p1t = const.tile([RB, m_core], bf16, name="p1t")
        nc.vector.memset(p1t[:], 1.0)

        # ---- helpers
        def xa_mm(mt):
            """(x@A) for one m-tile -> psum [128, 16]; returns psum tile."""
            pt = psxa.tile([P, r], f32, name="xa")
            for ki in range(KT):
                nc.tensor.matmul(pt[:], xslice(mt, ki), a_bf[:, ki, :],
                                 start=(ki == 0), stop=(ki == KT - 1))
            sb = xasb.tile([P, r], bf16, name="xasb")
            nc.vector.tensor_copy(sb[:], pt[:])
            return sb

        def xa_transpose(mt, sb):
            """PE-transpose (xA) [128,16] -> (xA)^T [16,128] into p1t."""
            tp = psxat.tile([r, P], bf16, name="xat")
            nc.tensor.transpose(tp[:], sb[:], identb[:])
            nc.vector.tensor_copy(p1t[0:r, mt * P:(mt + 1) * P], tp[:])

        def b_mm(ni, mt, ps):
            nsl = slice(ni * n_tile, (ni + 1) * n_tile)
            nc.tensor.matmul(ps[:], p1t[:, mt * P:(mt + 1) * P], b17[:, nsl],
                             start=False, stop=True)

        def drain(ni, mt, ps):
            nsl = slice(ni * n_tile, (ni + 1) * n_tile)
            ot = outstage.tile([P, n_tile], f32, name="ot")
            nc.scalar.activation(ot[:], ps[:], mybir.ActivationFunctionType.Copy)
            nc.scalar.dma_start(out[mt * P:(mt + 1) * P, nsl], ot[:])

        # ---- ni = 0: per-m-tile pipeline riding x arrival.
        # PE order per mt: [k-walk 0..16] [transpose(mt-1)] [k-walk 16..32]
        # [B-mm(mt-1)] [xa(mt)] -- the DVE copies for (mt-1)/(mt) run under
        # the k-walks so the PE never waits on them.
        x_emit = {2: 3, 4: 4, 5: 5, 6: 6, 7: 7, 8: 8}
        w_emit = {8 + i: i for i in range(8)}
        prev = None  # (mt, psum, xa_sb) of previous m-tile
        for mt in range(MT):
            if mt in x_emit:
                load_x(x_emit[mt])
            if mt in w_emit:
                load_wq(1, w_emit[mt])
            ps = psum.tile([P, n_tile], f32, name="mm")
            for ki in range(KT // 2):
                nc.tensor.matmul(ps[:], xslice(mt, ki), wslice(0, ki),
                                 start=(ki == 0), stop=False)
            if prev is not None:
                xa_transpose(prev[0], prev[2])
            for ki in range(KT // 2, KT):
                nc.tensor.matmul(ps[:], xslice(mt, ki), wslice(0, ki),
                                 start=False, stop=False)
            if prev is not None:
                b_mm(0, prev[0], prev[1])
                drain(0, prev[0], prev[1])
            sb = xa_mm(mt)
            prev = (mt, ps, sb)

        # ---- ni >= 1: groups of 4 m-tiles; finish ni=0's last m-tile
        # inside the first group of ni=1.
        def group(ni, m0, cnt, finish_prev=None):
            pss = [psum.tile([P, n_tile], f32, name="mm") for _ in range(cnt)]
            half = KT // 2
            for ki in range(half):
                for j in range(cnt):
                    nc.tensor.matmul(pss[j][:], xslice(m0 + j, ki),
                                     wslice(ni, ki), start=(ki == 0), stop=False)
            if finish_prev is not None:
                pmt, pps, psb = finish_prev
                xa_transpose(pmt, psb)
            for ki in range(half, KT):
                for j in range(cnt):
                    nc.tensor.matmul(pss[j][:], xslice(m0 + j, ki),
                                     wslice(ni, ki), start=False, stop=False)
            if finish_prev is not None:
                pmt, pps, psb = finish_prev
                b_mm(0, pmt, pps)
                drain(0, pmt, pps)
            for j in range(cnt):
                b_mm(ni, m0 + j, pss[j])
                drain(ni, m0 + j, pss[j])

        for ni in range(1, NT):
            if ni < NT - 1:
                for g, (m0, cnt) in enumerate([(0, 4), (4, 4), (8, 4), (12, 4)]):
                    load_wq(ni + 1, 2 * g)
                    load_wq(ni + 1, 2 * g + 1)
                    group(ni, m0, cnt,
                          finish_prev=prev if (ni == 1 and g == 0) else None)
            else:
                for m0, cnt in [(0, 4), (4, 4), (8, 4), (12, 2), (14, 1), (15, 1)]:
                    group(ni, m0, cnt)
    nc.compile()
    return nc


_CACHE = {}


def _get_nc(key, *args, **kw):
    if key not in _CACHE:
        _CACHE[key] = build_nc(*args, **kw)
    return _CACHE[key]


def kernel(x, W, bias, lora_A, lora_B, _trace=False):
    Bb, S, D = x.shape
    R = lora_A.shape[1]
    M = Bb * S
    m_core = M // NCORES
    nc = _get_nc(("v4", m_core, D, R), m_core, D, R)

    xf = np.asarray(x, dtype=np.float32).reshape(M, D)
    # [8, D, m_core]: per-core transposed x in one pass
    xT = np.ascontiguousarray(
        xf.reshape(NCORES, m_core, D).transpose(0, 2, 1))
    W = np.ascontiguousarray(W, dtype=np.float32)
    bias = np.ascontiguousarray(bias, dtype=np.float32)
    lora_A = np.ascontiguousarray(lora_A, dtype=np.float32)
    lora_B = np.ascontiguousarray(lora_B, dtype=np.float32)
    ident = np.eye(P, dtype=np.float32)

    in_maps = []
    for c in range(NCORES):
        in_maps.append({
            "xT": xT[c], "W": W, "bias": bias,
            "lora_A": lora_A, "lora_B": lora_B, "ident": ident,
        })
    res = run_bass_kernel_spmd(nc, in_maps, list(range(NCORES)), trace=_trace)
    outs = [res.results[c]["out"] for c in range(NCORES)]
    full = np.concatenate(outs, axis=0).reshape(Bb, S, D).astype(x.dtype)
    if _trace:
        return full, res
    return full


# revision 12
# speedup vs baseline: 1.0218x; 1.0218x over previous
"""HQQLinearLoRA TRN2 kernel v4: out = x @ W + (x @ A) @ B + bias.

Data-parallel over tokens (16384) across 8 cores; per core
[2048, 4096] @ [4096, 4096] + rank-16 LoRA + bias.

v4 design (cost-model driven):
- Host passes x already transposed (xT [d, m_core]) and an identity
  matrix; no on-device DMA transposes at all.
- All loads are SWDGE cast-DMAs (f32 DRAM -> bf16 SBUF): the DMA cost
  model charges destination bytes, so bf16 destinations halve transfer
  time, and no DVE/Act conversion passes exist.
- x^T resident in SBUF as 9 chunk tiles [128, 32, mw] bf16 (128 KB/part
  total); W streamed once as quarter-n-tile tiles [128, 8, 512] bf16.
- x@A via ap_size=16 matmuls (output [128m, 16]): ~7ns each on PE vs
  53ns for the [16, m] orientation; one PE transpose (vs identity) per
  m-tile flips the result into p1t [17, m] for the fused K=17
  LoRA+bias matmul (row 16 of p1t memset to 1.0 once).
- ni=0 walks m-tiles one at a time riding x arrival, software
  pipelined: transpose(mt-1) sits between the k-walk halves of mt and
  B-mm(mt-1) after the walk, so the PE stream never waits on the DVE
  copies. ni>=1 walks groups of 4 m-tiles (4 psum banks + 4 draining),
  W(ni+1) quarters prefetched one per group. Final n-tile tapers
  4/4/4/2/1/1 so the drain pipeline empties under the last matmuls.
"""
import numpy as np
from contextlib import ExitStack

import concourse.bass as bass
import concourse.tile as tile
import concourse.mybir as mybir
from concourse import bacc
from concourse.bass_utils import run_bass_kernel_spmd

P = 128
NCORES = 8

B_DIM, S_DIM, D_DIM, R_DIM = 4, 4096, 4096, 16


def build_nc(m_core, d, r, n_tile=512, N_WARM=76, N_FILL=13):
    KT = d // P                 # 32 k-tiles
    MT = m_core // P            # 16 m-tiles
    NT = d // n_tile            # 8 n-tiles
    QK = 4                      # k-tiles per W piece
    NQ = KT // QK               # 8 pieces per W n-tile
    f32 = mybir.dt.float32
    bf16 = mybir.dt.bfloat16
    RB = r + 1                  # fused lora+bias contraction depth

    # one x chunk per m-tile
    widths = [P] * MT
    starts = np.cumsum([0] + widths).tolist()

    def chunk_of(mt):
        return mt, 0

    nc = bacc.Bacc(target_bir_lowering=False)
    # xTd: per-m-tile x^T in [p, kt, m] contiguous order (host-shuffled)
    xTd = nc.declare_dram_parameter("xTd", [MT, P, KT, P], f32, isOutput=False)
    W = nc.declare_dram_parameter("W", [d, d], f32, isOutput=False)
    ab_d = nc.declare_dram_parameter("ab_d", [P, KT, r], f32, isOutput=False)
    b17_d = nc.declare_dram_parameter("b17_d", [r + 1, d], f32, isOutput=False)
    ident = nc.declare_dram_parameter("ident", [P, P], f32, isOutput=False)
    out = nc.declare_dram_parameter("out", [m_core, d], f32, isOutput=True)

    Wr = W.rearrange("(kt p) n -> p kt n", p=P)

    with tile.TileContext(nc) as tc, ExitStack() as ctx:
        const = ctx.enter_context(tc.tile_pool(name="const", bufs=1))
        xtp = ctx.enter_context(tc.tile_pool(name="xtp", bufs=1))
        wrp = ctx.enter_context(tc.tile_pool(name="wrp", bufs=12))
        xasb = ctx.enter_context(tc.tile_pool(name="xasb", bufs=2))
        outstage = ctx.enter_context(tc.tile_pool(name="outstage", bufs=3))
        psum = ctx.enter_context(tc.tile_pool(name="psum", bufs=6, space="PSUM"))
        psxa = ctx.enter_context(tc.tile_pool(name="psxa", bufs=1, space="PSUM"))
        psxat = ctx.enter_context(tc.tile_pool(name="psxat", bufs=1, space="PSUM"))

        # ---- resident x^T chunk tiles (cast-DMA'd on demand)
        xt_c = [xtp.tile([P, KT, w], bf16, name=f"xt{c}")
                for c, w in enumerate(widths)]

        def load_x(c):
            nc.gpsimd.dma_start(
                xt_c[c][:].rearrange("p kt m -> p (kt m)"),
                xTd[c].rearrange("p kt m -> p (kt m)"))

        def xslice(mt, ki):
            c, off = chunk_of(mt)
            return xt_c[c][:, ki, off:off + P]

        # ---- W quarter tiles, cast-DMA'd; wq[ni % 2][q] holds current set
        wq = {}

        def load_wq(ni, q):
            t = wrp.tile([P, QK, n_tile], bf16, name="wq")
            nsl = slice(ni * n_tile, (ni + 1) * n_tile)
            nc.gpsimd.dma_start(t[:], Wr[:, q * QK:(q + 1) * QK, nsl])
            wq[(ni, q)] = t

        def wslice(ni, ki):
            return wq[(ni, ki // QK)][:, ki % QK, :]

        # ---- preamble: emission order tuned so transfers land just in
        # time (xc0 + W0 gate the first k-walk; xc1 must land by mt1).
        a_bf = const.tile([P, KT, r], bf16, name="a_bf")
        identb = const.tile([P, P], bf16, name="identb")
        b17 = const.tile([RB, d], bf16, name="b17")
        warm = const.tile([P, P], bf16, name="warm")
        nc.vector.memset(warm[:], 0.25)

        def load_const(which):
            if which == "ab":
                nc.gpsimd.dma_start(a_bf[:].rearrange("p kt r -> p (kt r)"),
                                    ab_d.rearrange("p kt r -> p (kt r)"))
            elif which == "id":
                nc.gpsimd.dma_start(identb[:], ident[:, :])
            elif which == "b17":
                nc.gpsimd.dma_start(b17[:], b17_d[:, :])

        toks = [("x", 0), ("w", 0, 0), ("w", 0, 1), ("w", 0, 2), ("c", "ab"),
                ("w", 0, 3), ("w", 0, 4), ("x", 1), ("w", 0, 5), ("w", 0, 6),
                ("w", 0, 7), ("c", "id"), ("c", "b17")]
        toks += [("x", c) for c in range(2, MT)]
        for tok in toks:
            if tok[0] == "x":
                load_x(tok[1])
            elif tok[0] == "w":
                load_wq(tok[1], tok[2])
            else:
                load_const(tok[1])

        # warmup matmuls: junk work that holds the PE p-state ramp while
        # the first x/W transfers land; tuned to end as xc0+W0p0 arrive.
        junk = psxa.tile([P, P], f32, name="xa")
        for i in range(N_WARM):
            nc.tensor.matmul(junk[:], warm[:], warm[:],
                             start=(i == 0), stop=(i == N_WARM - 1))

        # ones row for the fused bias term: rows 0:16 are overwritten by
        # the per-m-tile copy2 writes before any B-matmul reads them.
        p1t = const.tile([RB, m_core], bf16, name="p1t")
        nc.vector.memset(p1t[:], 1.0)

        # ---- helpers
        def xa_mm(mt):
            """(x@A) for one m-tile -> psum [128, 16]; returns psum tile."""
            pt = psxa.tile([P, r], f32, name="xa")
            for ki in range(KT):
                nc.tensor.matmul(pt[:], xslice(mt, ki), a_bf[:, ki, :],
                                 start=(ki == 0), stop=(ki == KT - 1))
            sb = xasb.tile([P, r], bf16, name="xasb")
            nc.vector.tensor_copy(sb[:], pt[:])
            return sb

        def xa_transpose(mt, sb):
            """PE-transpose (xA) [128,16] -> (xA)^T [16,128] into p1t."""
            tp = psxat.tile([r, P], bf16, name="xat")
            nc.tensor.transpose(tp[:], sb[:], identb[:])
            nc.vector.tensor_copy(p1t[0:r, mt * P:(mt + 1) * P], tp[:])

        def b_mm(ni, mt, ps):
            nsl = slice(ni * n_tile, (ni + 1) * n_tile)
            nc.tensor.matmul(ps[:], p1t[:, mt * P:(mt + 1) * P], b17[:, nsl],
                             start=False, stop=True)

        def drain(ni, mt, ps):
            nsl = slice(ni * n_tile, (ni + 1) * n_tile)
            ot = outstage.tile([P, n_tile], f32, name="ot")
            nc.scalar.activation(ot[:], ps[:], mybir.ActivationFunctionType.Copy)
            nc.scalar.dma_start(out[mt * P:(mt + 1) * P, nsl], ot[:])

        # ---- ni = 0: per-m-tile pipeline riding x arrival.
        # PE order per mt: [k-walk 0..16] [transpose(mt-1)] [k-walk 16..32]
        # [B-mm(mt-1)] [xa(mt)] -- the DVE copies for (mt-1)/(mt) run under
        # the k-walks so the PE never waits on them.
        x_emit = {}
        w_emit = {8 + i: i for i in range(8)}
        prev = None  # (mt, psum, xa_sb) of previous m-tile
        for mt in range(MT):
            if mt in x_emit:
                load_x(x_emit[mt])
            if mt in w_emit:
                load_wq(1, w_emit[mt])
            ps = psum.tile([P, n_tile], f32, name="mm")
            fill = N_FILL if mt == 0 else 0
            for ki in range(KT // 2):
                if fill and ki and ki % QK == 0:
                    junk = psxa.tile([P, P], f32, name="xa")
                    for i in range(fill):
                        nc.tensor.matmul(junk[:], warm[:], warm[:],
                                         start=(i == 0), stop=(i == fill - 1))
                nc.tensor.matmul(ps[:], xslice(mt, ki), wslice(0, ki),
                                 start=(ki == 0), stop=False)
            if prev is not None:
                xa_transpose(prev[0], prev[2])
            for ki in range(KT // 2, KT):
                if fill and ki % QK == 0:
                    junk = psxa.tile([P, P], f32, name="xa")
                    for i in range(fill):
                        nc.tensor.matmul(junk[:], warm[:], warm[:],
                                         start=(i == 0), stop=(i == fill - 1))
                nc.tensor.matmul(ps[:], xslice(mt, ki), wslice(0, ki),
                                 start=False, stop=False)
            if prev is not None:
                b_mm(0, prev[0], prev[1])
                drain(0, prev[0], prev[1])
            sb = xa_mm(mt)
            prev = (mt, ps, sb)

        # ---- ni >= 1: groups of 4 m-tiles; finish ni=0's last m-tile
        # inside the first group of ni=1.
        def group(ni, m0, cnt, finish_prev=None):
            pss = [psum.tile([P, n_tile], f32, name="mm") for _ in range(cnt)]
            half = KT // 2
            for ki in range(half):
                for j in range(cnt):
                    nc.tensor.matmul(pss[j][:], xslice(m0 + j, ki),
                                     wslice(ni, ki), start=(ki == 0), stop=False)
            if finish_prev is not None:
                pmt, pps, psb = finish_prev
                xa_transpose(pmt, psb)
            for ki in range(half, KT):
                for j in range(cnt):
                    nc.tensor.matmul(pss[j][:], xslice(m0 + j, ki),
                                     wslice(ni, ki), start=False, stop=False)
            if finish_prev is not None:
                pmt, pps, psb = finish_prev
                b_mm(0, pmt, pps)
                drain(0, pmt, pps)
            for j in range(cnt):
                b_mm(ni, m0 + j, pss[j])
                drain(ni, m0 + j, pss[j])

        for ni in range(1, NT):
            if ni < NT - 1:
                for g, (m0, cnt) in enumerate([(0, 4), (4, 4), (8, 4), (12, 4)]):
                    load_wq(ni + 1, 2 * g)
                    load_wq(ni + 1, 2 * g + 1)
                    group(ni, m0, cnt,
                          finish_prev=prev if (ni == 1 and g == 0) else None)
            else:
                for m0, cnt in [(0, 4), (4, 4), (8, 4), (12, 2), (14, 1)]:
                    group(ni, m0, cnt)
                # final m-tile: 256+128+128 psum pieces; earlier pieces
                # drain on Act under the later k-walks, the last piece
                # drains through idle DVE+SP for the shortest tail
                mt = MT - 1
                pieces = [(0, 256, "act"), (256, 128, "sp"), (384, 128, "act2")]
                for off, wid, eng in pieces:
                    nsl = slice(ni * n_tile + off, ni * n_tile + off + wid)
                    ph = psum.tile([P, wid], f32, name="mm")
                    for ki in range(KT):
                        nc.tensor.matmul(
                            ph[:], xslice(mt, ki),
                            wq[(ni, ki // QK)][:, ki % QK, off:off + wid],
                            start=(ki == 0), stop=False)
                    nc.tensor.matmul(
                        ph[:], p1t[:, mt * P:(mt + 1) * P], b17[:, nsl],
                        start=False, stop=True)
                    ot = outstage.tile([P, wid], f32, name="ot")
                    if eng == "act":
                        nc.scalar.activation(ot[:], ph[:],
                                             mybir.ActivationFunctionType.Copy)
                        nc.scalar.dma_start(out[mt * P:(mt + 1) * P, nsl], ot[:])
                    elif eng == "sp":
                        nc.vector.tensor_copy(ot[:], ph[:])
                        nc.sync.dma_start(out[mt * P:(mt + 1) * P, nsl], ot[:])
                    else:
                        nc.scalar.activation(ot[:], ph[:],
                                             mybir.ActivationFunctionType.Copy)
                        nc.scalar.dma_start(out[mt * P:(mt + 1) * P, nsl], ot[:])
    nc.compile()
    return nc


_CACHE = {}


def _get_nc(key, *args, **kw):
    if key not in _CACHE:
        _CACHE[key] = build_nc(*args, **kw)
    return _CACHE[key]


def kernel(x, W, bias, lora_A, lora_B, _trace=False):
    Bb, S, D = x.shape
    R = lora_A.shape[1]
    M = Bb * S
    m_core = M // NCORES
    nc = _get_nc(("v4", m_core, D, R), m_core, D, R)

    MT, KT = m_core // P, D // P
    xf = np.asarray(x, dtype=np.float32).reshape(M, D)
    # [core, mt, p(k%128), kt, m%128]: chunk-contiguous transposed x
    xTd = np.ascontiguousarray(
        xf.reshape(NCORES, MT, P, KT, P).transpose(0, 1, 4, 3, 2))
    W = np.ascontiguousarray(W, dtype=np.float32)
    ab_d = np.ascontiguousarray(
        np.asarray(lora_A, dtype=np.float32).reshape(KT, P, R).transpose(1, 0, 2))
    b17_d = np.ascontiguousarray(np.concatenate(
        [np.asarray(lora_B, dtype=np.float32),
         np.asarray(bias, dtype=np.float32).reshape(1, D)], axis=0))
    ident = np.eye(P, dtype=np.float32)

    in_maps = []
    for c in range(NCORES):
        in_maps.append({
            "xTd": xTd[c], "W": W, "ab_d": ab_d, "b17_d": b17_d,
            "ident": ident,
        })
    res = run_bass_kernel_spmd(nc, in_maps, list(range(NCORES)), trace=_trace)
    outs = [res.results[c]["out"] for c in range(NCORES)]
    full = np.concatenate(outs, axis=0).reshape(Bb, S, D).astype(x.dtype)
    if _trace:
        return full, res
    return full


# revision 16
# speedup vs baseline: 1.0224x; 1.0006x over previous
"""HQQLinearLoRA TRN2 kernel v4: out = x @ W + (x @ A) @ B + bias.

Data-parallel over tokens (16384) across 8 cores; per core
[2048, 4096] @ [4096, 4096] + rank-16 LoRA + bias.

v4 design (cost-model driven):
- Host passes x already transposed (xT [d, m_core]) and an identity
  matrix; no on-device DMA transposes at all.
- All loads are SWDGE cast-DMAs (f32 DRAM -> bf16 SBUF): the DMA cost
  model charges destination bytes, so bf16 destinations halve transfer
  time, and no DVE/Act conversion passes exist.
- x^T resident in SBUF as 9 chunk tiles [128, 32, mw] bf16 (128 KB/part
  total); W streamed once as quarter-n-tile tiles [128, 8, 512] bf16.
- x@A via ap_size=16 matmuls (output [128m, 16]): ~7ns each on PE vs
  53ns for the [16, m] orientation; one PE transpose (vs identity) per
  m-tile flips the result into p1t [17, m] for the fused K=17
  LoRA+bias matmul (row 16 of p1t memset to 1.0 once).
- ni=0 walks m-tiles one at a time riding x arrival, software
  pipelined: transpose(mt-1) sits between the k-walk halves of mt and
  B-mm(mt-1) after the walk, so the PE stream never waits on the DVE
  copies. ni>=1 walks groups of 4 m-tiles (4 psum banks + 4 draining),
  W(ni+1) quarters prefetched one per group. Final n-tile tapers
  4/4/4/2/1/1 so the drain pipeline empties under the last matmuls.
"""
import numpy as np
from contextlib import ExitStack

import concourse.bass as bass
import concourse.tile as tile
import concourse.mybir as mybir
from concourse import bacc
from concourse.bass_utils import run_bass_kernel_spmd

P = 128
NCORES = 8

B_DIM, S_DIM, D_DIM, R_DIM = 4, 4096, 4096, 16


def build_nc(m_core, d, r, n_tile=512, N_WARM=69, N_FILL=13):
    KT = d // P                 # 32 k-tiles
    MT = m_core // P            # 16 m-tiles
    NT = d // n_tile            # 8 n-tiles
    QK = 4                      # k-tiles per W piece
    NQ = KT // QK               # 8 pieces per W n-tile
    f32 = mybir.dt.float32
    bf16 = mybir.dt.bfloat16
    RB = r + 1                  # fused lora+bias contraction depth

    # one x chunk per m-tile
    widths = [P] * MT
    starts = np.cumsum([0] + widths).tolist()

    def chunk_of(mt):
        return mt, 0

    nc = bacc.Bacc(target_bir_lowering=False)
    # xTd: per-m-tile x^T in [p, kt, m] contiguous order (host-shuffled)
    xTd = nc.declare_dram_parameter("xTd", [MT, P, KT, P], f32, isOutput=False)
    W = nc.declare_dram_parameter("W", [d, d], f32, isOutput=False)
    ab_d = nc.declare_dram_parameter("ab_d", [P, KT, r], f32, isOutput=False)
    b17_d = nc.declare_dram_parameter("b17_d", [r + 1, d], f32, isOutput=False)
    ident = nc.declare_dram_parameter("ident", [P, P], f32, isOutput=False)
    out = nc.declare_dram_parameter("out", [m_core, d], f32, isOutput=True)

    Wr = W.rearrange("(kt p) n -> p kt n", p=P)

    with tile.TileContext(nc) as tc, ExitStack() as ctx:
        const = ctx.enter_context(tc.tile_pool(name="const", bufs=1))
        xtp = ctx.enter_context(tc.tile_pool(name="xtp", bufs=1))
        wrp = ctx.enter_context(tc.tile_pool(name="wrp", bufs=12))
        xasb = ctx.enter_context(tc.tile_pool(name="xasb", bufs=2))
        outstage = ctx.enter_context(tc.tile_pool(name="outstage", bufs=3))
        psum = ctx.enter_context(tc.tile_pool(name="psum", bufs=6, space="PSUM"))
        psxa = ctx.enter_context(tc.tile_pool(name="psxa", bufs=1, space="PSUM"))
        psxat = ctx.enter_context(tc.tile_pool(name="psxat", bufs=1, space="PSUM"))

        # ---- resident x^T chunk tiles (cast-DMA'd on demand)
        xt_c = [xtp.tile([P, KT, w], bf16, name=f"xt{c}")
                for c, w in enumerate(widths)]

        def load_x(c):
            nc.gpsimd.dma_start(
                xt_c[c][:].rearrange("p kt m -> p (kt m)"),
                xTd[c].rearrange("p kt m -> p (kt m)"))

        def xslice(mt, ki):
            c, off = chunk_of(mt)
            return xt_c[c][:, ki, off:off + P]

        # ---- W quarter tiles, cast-DMA'd; wq[ni % 2][q] holds current set
        wq = {}

        def load_wq(ni, q):
            t = wrp.tile([P, QK, n_tile], bf16, name="wq")
            nsl = slice(ni * n_tile, (ni + 1) * n_tile)
            nc.gpsimd.dma_start(t[:], Wr[:, q * QK:(q + 1) * QK, nsl])
            wq[(ni, q)] = t

        def wslice(ni, ki):
            return wq[(ni, ki // QK)][:, ki % QK, :]

        # ---- preamble: emission order tuned so transfers land just in
        # time (xc0 + W0 gate the first k-walk; xc1 must land by mt1).
        a_bf = const.tile([P, KT, r], bf16, name="a_bf")
        identb = const.tile([P, P], bf16, name="identb")
        b17 = const.tile([RB, d], bf16, name="b17")
        warm = const.tile([P, P], bf16, name="warm")
        nc.vector.memset(warm[:], 0.25)

        def load_const(which):
            if which == "ab":
                nc.gpsimd.dma_start(a_bf[:].rearrange("p kt r -> p (kt r)"),
                                    ab_d.rearrange("p kt r -> p (kt r)"))
            elif which == "id":
                nc.gpsimd.dma_start(identb[:], ident[:, :])
            elif which == "b17":
                nc.gpsimd.dma_start(b17[:], b17_d[:, :])

        toks = [("x", 0), ("w", 0, 0), ("w", 0, 1), ("w", 0, 2), ("c", "ab"),
                ("c", "id"), ("w", 0, 3), ("w", 0, 4), ("x", 1), ("w", 0, 5),
                ("w", 0, 6), ("w", 0, 7), ("x", 2), ("c", "b17")]
        toks += [("x", c) for c in range(3, MT)]
        for tok in toks:
            if tok[0] == "x":
                load_x(tok[1])
            elif tok[0] == "w":
                load_wq(tok[1], tok[2])
            else:
                load_const(tok[1])

        # warmup matmuls: junk work that holds the PE p-state ramp while
        # the first x/W transfers land; tuned to end as xc0+W0p0 arrive.
        junk = psxa.tile([P, P], f32, name="xa")
        for i in range(N_WARM):
            nc.tensor.matmul(junk[:], warm[:], warm[:],
                             start=(i == 0), stop=(i == N_WARM - 1))

        # ones row for the fused bias term: rows 0:16 are overwritten by
        # the per-m-tile copy2 writes before any B-matmul reads them.
        p1t = const.tile([RB, m_core], bf16, name="p1t")
        nc.vector.memset(p1t[:], 1.0)

        # ---- helpers
        def xa_mm(mt):
            """(x@A) for one m-tile -> psum [128, 16]; returns psum tile."""
            pt = psxa.tile([P, r], f32, name="xa")
            for ki in range(KT):
                nc.tensor.matmul(pt[:], xslice(mt, ki), a_bf[:, ki, :],
                                 start=(ki == 0), stop=(ki == KT - 1))
            sb = xasb.tile([P, r], bf16, name="xasb")
            nc.vector.tensor_copy(sb[:], pt[:])
            return sb

        def xa_transpose(mt, sb):
            """PE-transpose (xA) [128,16] -> (xA)^T [16,128] into p1t."""
            tp = psxat.tile([r, P], bf16, name="xat")
            nc.tensor.transpose(tp[:], sb[:], identb[:])
            nc.vector.tensor_copy(p1t[0:r, mt * P:(mt + 1) * P], tp[:])

        def b_mm(ni, mt, ps):
            nsl = slice(ni * n_tile, (ni + 1) * n_tile)
            nc.tensor.matmul(ps[:], p1t[:, mt * P:(mt + 1) * P], b17[:, nsl],
                             start=False, stop=True)

        def drain(ni, mt, ps):
            nsl = slice(ni * n_tile, (ni + 1) * n_tile)
            ot = outstage.tile([P, n_tile], f32, name="ot")
            nc.scalar.activation(ot[:], ps[:], mybir.ActivationFunctionType.Copy)
            nc.scalar.dma_start(out[mt * P:(mt + 1) * P, nsl], ot[:])

        # ---- ni = 0: per-m-tile pipeline riding x arrival.
        # PE order per mt: [k-walk 0..16] [transpose(mt-1)] [k-walk 16..32]
        # [B-mm(mt-1)] [xa(mt)] -- the DVE copies for (mt-1)/(mt) run under
        # the k-walks so the PE never waits on them.
        x_emit = {}
        w_emit = {8 + i: i for i in range(8)}
        prev = None  # (mt, psum, xa_sb) of previous m-tile
        for mt in range(MT):
            if mt in x_emit:
                load_x(x_emit[mt])
            if mt in w_emit:
                load_wq(1, w_emit[mt])
            ps = psum.tile([P, n_tile], f32, name="mm")
            fill = N_FILL if mt == 0 else 0
            for ki in range(KT // 2):
                if fill and ki and ki % QK == 0:
                    junk = psxa.tile([P, P], f32, name="xa")
                    for i in range(fill):
                        nc.tensor.matmul(junk[:], warm[:], warm[:],
                                         start=(i == 0), stop=(i == fill - 1))
                nc.tensor.matmul(ps[:], xslice(mt, ki), wslice(0, ki),
                                 start=(ki == 0), stop=False)
            if prev is not None:
                xa_transpose(prev[0], prev[2])
            for ki in range(KT // 2, KT):
                if fill and ki % QK == 0:
                    junk = psxa.tile([P, P], f32, name="xa")
                    for i in range(fill):
                        nc.tensor.matmul(junk[:], warm[:], warm[:],
                                         start=(i == 0), stop=(i == fill - 1))
                nc.tensor.matmul(ps[:], xslice(mt, ki), wslice(0, ki),
                                 start=False, stop=False)
            if prev is not None:
                b_mm(0, prev[0], prev[1])
                drain(0, prev[0], prev[1])
            sb = xa_mm(mt)
            prev = (mt, ps, sb)

        # ---- ni >= 1: groups of 4 m-tiles; finish ni=0's last m-tile
        # inside the first group of ni=1.
        def group(ni, m0, cnt, finish_prev=None):
            pss = [psum.tile([P, n_tile], f32, name="mm") for _ in range(cnt)]
            half = KT // 2
            for ki in range(half):
                for j in range(cnt):
                    nc.tensor.matmul(pss[j][:], xslice(m0 + j, ki),
                                     wslice(ni, ki), start=(ki == 0), stop=False)
            if finish_prev is not None:
                pmt, pps, psb = finish_prev
                xa_transpose(pmt, psb)
            for ki in range(half, KT):
                for j in range(cnt):
                    nc.tensor.matmul(pss[j][:], xslice(m0 + j, ki),
                                     wslice(ni, ki), start=False, stop=False)
            if finish_prev is not None:
                pmt, pps, psb = finish_prev
                b_mm(0, pmt, pps)
                drain(0, pmt, pps)
            for j in range(cnt):
                b_mm(ni, m0 + j, pss[j])
                drain(ni, m0 + j, pss[j])

        for ni in range(1, NT):
            if ni < NT - 1:
                for g, (m0, cnt) in enumerate([(0, 4), (4, 4), (8, 4), (12, 4)]):
                    load_wq(ni + 1, 2 * g)
                    load_wq(ni + 1, 2 * g + 1)
                    group(ni, m0, cnt,
                          finish_prev=prev if (ni == 1 and g == 0) else None)
            else:
                for m0, cnt in [(0, 4), (4, 4), (8, 4), (12, 2), (14, 1)]:
                    group(ni, m0, cnt)
                # final m-tile: 256+128+128 psum pieces; earlier pieces
                # drain on Act under the later k-walks, the last piece
                # drains through idle DVE+SP for the shortest tail
                mt = MT - 1
                pieces = [(0, 256, "act"), (256, 128, "sp"), (384, 128, "act2")]
                for off, wid, eng in pieces:
                    nsl = slice(ni * n_tile + off, ni * n_tile + off + wid)
                    ph = psum.tile([P, wid], f32, name="mm")
                    for ki in range(KT):
                        nc.tensor.matmul(
                            ph[:], xslice(mt, ki),
                            wq[(ni, ki // QK)][:, ki % QK, off:off + wid],
                            start=(ki == 0), stop=False)
                    nc.tensor.matmul(
                        ph[:], p1t[:, mt * P:(mt + 1) * P], b17[:, nsl],
                        start=False, stop=True)
                    ot = outstage.tile([P, wid], f32, name="ot")
                    if eng == "act":
                        nc.scalar.activation(ot[:], ph[:],
                                             mybir.ActivationFunctionType.Copy)
                        nc.scalar.dma_start(out[mt * P:(mt + 1) * P, nsl], ot[:])
                    elif eng == "sp":
                        nc.vector.tensor_copy(ot[:], ph[:])
                        nc.sync.dma_start(out[mt * P:(mt + 1) * P, nsl], ot[:])
                    else:
                        nc.scalar.activation(ot[:], ph[:],
                                             mybir.ActivationFunctionType.Copy)
                        nc.scalar.dma_start(out[mt * P:(mt + 1) * P, nsl], ot[:])
    nc.compile()
    return nc


_CACHE = {}


def _get_nc(key, *args, **kw):
    if key not in _CACHE:
        _CACHE[key] = build_nc(*args, **kw)
    return _CACHE[key]


def kernel(x, W, bias, lora_A, lora_B, _trace=False):
    Bb, S, D = x.shape
    R = lora_A.shape[1]
    M = Bb * S
    m_core = M // NCORES
    nc = _get_nc(("v4", m_core, D, R), m_core, D, R)

    MT, KT = m_core // P, D // P
    xf = np.asarray(x, dtype=np.float32).reshape(M, D)
    # [core, mt, p(k%128), kt, m%128]: chunk-contiguous transposed x
    xTd = np.ascontiguousarray(
        xf.reshape(NCORES, MT, P, KT, P).transpose(0, 1, 4, 3, 2))
    W = np.ascontiguousarray(W, dtype=np.float32)
    ab_d = np.ascontiguousarray(
        np.asarray(lora_A, dtype=np.float32).reshape(KT, P, R).transpose(1, 0, 2))
    b17_d = np.ascontiguousarray(np.concatenate(
        [np.asarray(lora_B, dtype=np.float32),
         np.asarray(bias, dtype=np.float32).reshape(1, D)], axis=0))
    ident = np.eye(P, dtype=np.float32)

    in_maps = []
    for c in range(NCORES):
        in_maps.append({
            "xTd": xTd[c], "W": W, "ab_d": ab_d, "b17_d": b17_d,
            "ident": ident,
        })
    res = run_bass_kernel_spmd(nc, in_maps, list(range(NCORES)), trace=_trace)
    outs = [res.results[c]["out"] for c in range(NCORES)]
    full = np.concatenate(outs, axis=0).reshape(Bb, S, D).astype(x.dtype)
    if _trace:
        return full, res
    return full
